# revision 1
# baseline (speedup 1.0000x reference)
"""Multi-head attention block (B=4, N=2048, C=1024, H=16) on 8 TRN2 NeuronCores.

Sharding (SPMD, no collectives): core i handles batch b = i//2 and heads
[8*(i%2), 8*(i%2)+8) -- data parallel over batch x tensor parallel over heads.
Host side: compacts keys to the ~50% with mask==1 (reference's masked softmax
terms are exactly 0 in fp32), zero-pads to KP=1152, transposes x, slices and
casts weights to bf16. The two per-batch partial projections are summed on the
host (the tensor-parallel all-reduce) and b_proj added.

Default device kernel (build_kernel_v3, opts in DEFAULT_OPTS; all matmuls
bf16 with fp32 PSUM):
  1. Interleaved schedule: the q/k/v projection chains run as PE gap-fillers
     INSIDE the per-head attention loop (vproj at head 0; per-pair q/k
     chains at heads 1-5) so the in-order PE never starves while the
     Activation engine computes exps.  x^T stays SBUF-resident.
  2. Per head: scores^T = K Q^T into 3-buffered [128,1024] PSUM tiles;
     ACT computes exp(0.125*s + bias) from PSUM (bias carries the pad mask).
  3. AV (form B): stationary V block per head is 128 wide -- 64 value
     columns plus 64 REPLICATED ones columns, so each AV matmul emits U^T
     on partitions 0-63 and the softmax denominator broadcast across
     partitions 64-127 at zero extra PE cycles.  Normalization is then one
     full-width [64,512] reciprocal + multiply per 512-query chunk on
     VectorE, straight out of PSUM.  No DRAM bounce, no single-partition
     DVE ops (both are brutally slow on real HW), no custom GPSIMD ops
     (absent from this bedrock image).
  4. Output projection from per-pair uT tiles; PSUM evicted on ACT (idle
     tail); partial written to DRAM in bf16 (summed in fp32 on the host).

The fillerless last head-pair is software-pipelined (head 7 scores
interleave with head 6 AV chunks, "lacell").

Measured: ~310-390 us per pass on HW (machine-state drift dominates;
sim/cost-model 237 us vs 291 us for the previous default); absmax
relative error ~7e-3 (bf16-limited).
Profiling notes: CoreSim timing-only sim (no_exec) matches v1 HW within
5%; the big HW/sim divergences are single-partition DVE/DMA ops and
custom GPSIMD library ops -- see memory notes.
"""
import os
import time

import numpy as np
import ml_dtypes

import concourse.bass as bass
import concourse.mybir as mybir
import concourse.tile as tile
from concourse import bacc
from concourse.masks import make_identity

B, N, C, H, HD = 4, 2048, 1024, 16, 64
KP = 1152          # compacted+padded key count (9 tiles of 128)
NKT = KP // 128    # 9 key tiles
HPC = 8            # heads per core
MPC = HPC * HD     # 512 = qkv columns per core
NQT = N // 128     # 16 query tiles
BF = mybir.dt.bfloat16
F32 = mybir.dt.float32
bfloat16 = ml_dtypes.bfloat16

VERBOSE = bool(int(os.environ.get("KERNEL_VERBOSE", "0")))

_compiled = {}


def _log(msg):
    if VERBOSE:
        print(f"[kernel] {msg}", flush=True)


def build_kernel(reps=1, stop_after=None, opts=()):
    if "v3" in opts:
        return build_kernel_v3(reps=reps, opts=opts)
    opts = set(opts)
    nc = bacc.Bacc("TRN2", num_devices=8)
    xT = nc.dram_tensor("xT", [C, N], BF, kind="ExternalInput")
    xcT = nc.dram_tensor("xcT", [C, KP], BF, kind="ExternalInput")
    wq = nc.dram_tensor("wq", [C, MPC], BF, kind="ExternalInput")
    wk = nc.dram_tensor("wk", [C, MPC], BF, kind="ExternalInput")
    wv = nc.dram_tensor("wv", [C, MPC], BF, kind="ExternalInput")
    wp = nc.dram_tensor("wp", [MPC, C], BF, kind="ExternalInput")
    biasv = nc.dram_tensor("biasv", [KP], F32, kind="ExternalInput")
    partial = nc.dram_tensor("partial", [N, C], F32, kind="ExternalOutput")

    KC = C // 128  # 8 contraction tiles over C

    import contextlib

    with tile.TileContext(nc) as tc:
        with contextlib.ExitStack() as stack:
            persist = stack.enter_context(tc.tile_pool(name="persist", bufs=1))
            xtp = stack.enter_context(tc.tile_pool(
                name="xtp", bufs=8 if ("e13" in opts or "e14" in opts) else 10))
            expp = stack.enter_context(tc.tile_pool(
                name="exps", bufs=(15 if "e15" in opts else
                                   14 if "e14" in opts else
                                   13 if "e13" in opts else 11)))
            small = stack.enter_context(tc.tile_pool(name="small", bufs=4))
            ostage = stack.enter_context(tc.tile_pool(name="ostage", bufs=3))
            if opts & {"formbn", "formbn2"}:
                nstage = stack.enter_context(tc.tile_pool(name="nstage", bufs=2))
                bcastp = stack.enter_context(tc.tile_pool(name="bcastp", bufs=2))
                dramp = stack.enter_context(
                    tc.tile_pool(name="dramp", bufs=3, space="DRAM"))
            av_bufs = 4 if (("avi" in opts and "avi_sep" not in opts
                             and "s3" not in opts)
                            or "formb2" in opts) else 2
            ps_s = stack.enter_context(
                tc.tile_pool(name="ps_s", bufs=3 if "s3" in opts else 2,
                             space="PSUM"))
            ps_av = stack.enter_context(
                tc.tile_pool(name="ps_av", bufs=av_bufs, space="PSUM"))
            if "dmat" in opts:
                ps_t = None
            elif "s3" in opts:
                ps_t = ps_av
            elif "avi_sep" in opts:
                ps_t = stack.enter_context(
                    tc.tile_pool(name="ps_t", bufs=2, space="PSUM"))
            elif "avi" in opts or "formb2" in opts:
                ps_t = None  # transposes use ps_s pool (tag "s")
            else:
                ps_t = stack.enter_context(
                    tc.tile_pool(name="ps_t", bufs=2, space="PSUM"))
            if reps > 1:
                hints = ((mybir.EngineType.PE, mybir.EngineType.Activation,
                          mybir.EngineType.DVE, mybir.EngineType.Pool,
                          mybir.EngineType.SP)
                         if "hint" in opts else ())
                stack.enter_context(tc.For_i(0, reps, 1, hint_engines=hints))
            # ---- persistent SBUF tensors ----
            if "wsplit" in opts:
                wqg = [persist.tile([128, MPC], BF, tag=f"wq{kc}",
                                    name=f"wq{kc}") for kc in range(KC)]
                wkg = [persist.tile([128, MPC], BF, tag=f"wk{kc}",
                                    name=f"wk{kc}") for kc in range(KC)]
                wvg = [persist.tile([128, MPC], BF, tag=f"wv{kc}",
                                    name=f"wv{kc}") for kc in range(KC)]
            else:
                wq_sb = persist.tile([128, KC * MPC], BF, tag="wq")
                wk_sb = persist.tile([128, KC * MPC], BF, tag="wk")
                wv_sb = persist.tile([128, KC * MPC], BF, tag="wv")
            wp_sb = persist.tile([128, 4 * C], BF, tag="wp")
            if "xsplit" in opts:
                xcg = [persist.tile([128, KP], BF, tag=f"xc{kc}",
                                    name=f"xc{kc}") for kc in range(KC)]
            else:
                xcT_sb = persist.tile([128, KC * KP], BF, tag="xcT")
            qT_sb = persist.tile([128, 4 * N], BF, tag="qT")      # head h: part (h%2)*64, col (h//2)*N
            kT_sb = persist.tile([128, 4 * KP], BF, tag="kT")     # head h: part (h%2)*64, col (h//2)*KP
            v_sb = persist.tile([128, NKT * (HPC * (HD + 1))], BF, tag="v")
            formb_mode = bool(opts & {"formb", "formb2", "formbn", "formbn2"})
            if not formb_mode:
                u_sb = persist.tile([128, NQT * MPC], BF, tag="u")
            uT_sb = persist.tile([128, 4 * N], BF, tag="uT")      # [c-part, ct*N + q]
            bias_sb = persist.tile([128, NKT], F32, tag="bias")
            if not formb_mode and "dmat" not in opts:
                ident_sb = persist.tile([128, 128], BF, tag="ident")
                make_identity(nc, ident_sb[:])

            # ---- input DMAs ----
            if "wsplit" in opts:
                # gating order: wk/xc chunk pairs first, then wv, then wq
                for kc in range(KC):
                    nc.sync.dma_start(out=wkg[kc][:],
                                      in_=wk[kc * 128:(kc + 1) * 128, :])
                    if "xsplit" in opts:
                        nc.sync.dma_start(
                            out=xcg[kc][:],
                            in_=xcT[kc * 128:(kc + 1) * 128, :])
                for kc in range(KC):
                    nc.sync.dma_start(out=wvg[kc][:],
                                      in_=wv[kc * 128:(kc + 1) * 128, :])
                for kc in range(KC):
                    nc.sync.dma_start(out=wqg[kc][:],
                                      in_=wq[kc * 128:(kc + 1) * 128, :])
            else:
                nc.sync.dma_start(
                    out=wq_sb[:].rearrange("p (kc m) -> p kc m", kc=KC),
                    in_=wq.rearrange("(kc p) m -> p kc m", p=128))
                nc.sync.dma_start(
                    out=wk_sb[:].rearrange("p (kc m) -> p kc m", kc=KC),
                    in_=wk.rearrange("(kc p) m -> p kc m", p=128))
                nc.sync.dma_start(
                    out=wv_sb[:].rearrange("p (kc m) -> p kc m", kc=KC),
                    in_=wv.rearrange("(kc p) m -> p kc m", p=128))
            nc.sync.dma_start(
                out=wp_sb[:].rearrange("p (kc m) -> p kc m", kc=KC if "wp" != "wp" else 4),
                in_=wp.rearrange("(kc p) m -> p kc m", p=128))
            if "xsplit" in opts:
                if "wsplit" not in opts:
                    for kc in range(KC):
                        nc.sync.dma_start(
                            out=xcg[kc][:],
                            in_=xcT[kc * 128:(kc + 1) * 128, :])
            else:
                nc.sync.dma_start(
                    out=xcT_sb[:].rearrange("p (kc k) -> p kc k", kc=KC),
                    in_=xcT.rearrange("(kc p) k -> p kc k", p=128))
            nc.sync.dma_start(
                out=bias_sb[:], in_=biasv.rearrange("(kt p) -> p kt", p=128))

            # ones column in v_sb (softmax denominators): col kt*520 + h*65 + 64
            for kt in range(NKT):
                nc.vector.memset(
                    v_sb[:, kt * (HPC * 65) + 64: (kt + 1) * (HPC * 65): 65], 1.0)

            # ---- qkv projections ----
            # k^T = Wk^T @ xc^T : [MPC, KP], packed per head-pair
            for mt in range(MPC // 128):
                for qc in range(KP // 384):
                    pk = ps_s.tile([128, 384], F32, tag="s")
                    for kc in range(KC):
                        nc.tensor.matmul(
                            pk[:],
                            (wkg[kc][:, mt * 128:(mt + 1) * 128]
                             if "wsplit" in opts else
                             wk_sb[:, kc * MPC + mt * 128: kc * MPC + (mt + 1) * 128]),
                            (xcg[kc][:, qc * 384:(qc + 1) * 384]
                             if "xsplit" in opts else
                             xcT_sb[:, kc * KP + qc * 384: kc * KP + (qc + 1) * 384]),
                            start=(kc == 0), stop=(kc == KC - 1))
                    nc.vector.tensor_copy(
                        kT_sb[:, mt * KP + qc * 384: mt * KP + (qc + 1) * 384], pk[:])

            # v = xc @ Wv : [KP, MPC], interleaved with ones columns
            for kt in range(NKT):
                pv = ps_s.tile([128, MPC], F32, tag="s")
                for kc in range(KC):
                    nc.tensor.matmul(
                        pv[:],
                        (xcg[kc][:, kt * 128:(kt + 1) * 128]
                         if "xsplit" in opts else
                         xcT_sb[:, kc * KP + kt * 128: kc * KP + (kt + 1) * 128]),
                        (wvg[kc][:] if "wsplit" in opts else
                         wv_sb[:, kc * MPC: (kc + 1) * MPC]),
                        start=(kc == 0), stop=(kc == KC - 1))
                # scatter heads into 65-strided layout
                vdst = v_sb[:, kt * (HPC * 65): (kt + 1) * (HPC * 65)]
                vdst3 = vdst.rearrange("p (h d) -> p h d", h=HPC)[:, :, 0:HD]
                psrc3 = pv.rearrange("p (h d) -> p h d", h=HPC)
                nc.vector.tensor_copy(vdst3, psrc3)

            # q^T = Wq^T @ x^T : [MPC, N], packed per head-pair
            for qc in range(N // 512):
                xt_tiles = []
                for kc in range(KC):
                    xt = xtp.tile([128, 512], BF)
                    nc.sync.dma_start(
                        out=xt[:], in_=xT[kc * 128:(kc + 1) * 128,
                                          qc * 512:(qc + 1) * 512])
                    xt_tiles.append(xt)
                for mt in range(MPC // 128):
                    pq = ps_s.tile([128, 512], F32, tag="s")
                    for kc in range(KC):
                        nc.tensor.matmul(
                            pq[:],
                            (wqg[kc][:, mt * 128:(mt + 1) * 128]
                             if "wsplit" in opts else
                             wq_sb[:, kc * MPC + mt * 128: kc * MPC + (mt + 1) * 128]),
                            xt_tiles[kc][:],
                            start=(kc == 0), stop=(kc == KC - 1))
                    nc.vector.tensor_copy(
                        qT_sb[:, mt * N + qc * 512: mt * N + (qc + 1) * 512], pq[:])

            # ---- attention per head ----
            for h in range(HPC) if stop_after != "qkv" else []:
                po = (h % 2) * 64
                kcol = (h // 2) * KP
                qcol = (h // 2) * N
                exp_tiles = []
                for kt in range(NKT):
                    et = expp.tile([128, N], BF)
                    for qh in range(2):
                        ps = ps_s.tile([128, 1024], F32, tag="s")
                        for q2 in range(2):
                            nc.tensor.matmul(
                                ps[:, q2 * 512:(q2 + 1) * 512],
                                kT_sb[po:po + 64,
                                      kcol + kt * 128: kcol + (kt + 1) * 128],
                                qT_sb[po:po + 64,
                                      qcol + qh * 1024 + q2 * 512:
                                      qcol + qh * 1024 + (q2 + 1) * 512],
                                start=True, stop=True)
                        nc.scalar.activation(
                            et[:, qh * 1024:(qh + 1) * 1024], ps[:],
                            mybir.ActivationFunctionType.Exp,
                            bias=bias_sb[:, kt:kt + 1], scale=0.125)
                    exp_tiles.append(et)

                if "avi" in opts or "avi_sep" in opts:
                    # 2-way interleaved AV accumulation chains
                    for qp in range(NQT // 2) if stop_after not in ("qkv", "exp") else []:
                        pavs = [ps_av.tile([128, HD + 1], F32, tag="avt",
                                            name=f"pav{j}")
                                for j in range(2)]
                        for kt in range(NKT):
                            for j in range(2):
                                qt = qp * 2 + j
                                nc.tensor.matmul(
                                    pavs[j][:],
                                    exp_tiles[kt][:, qt * 128:(qt + 1) * 128],
                                    v_sb[:, kt * (HPC * 65) + h * 65:
                                         kt * (HPC * 65) + (h + 1) * 65],
                                    start=(kt == 0), stop=(kt == NKT - 1))
                        for j in range(2):
                            qt = qp * 2 + j
                            rcp = small.tile([128, 1], F32)
                            nc.vector.reciprocal(rcp[:], pavs[j][:, HD:HD + 1])
                            nc.vector.tensor_scalar_mul(
                                u_sb[:, qt * MPC + h * HD: qt * MPC + (h + 1) * HD],
                                pavs[j][:, 0:HD], rcp[:])
                elif "formbn2" in opts:
                    # per-chunk: AV -> sums copy -> fold -> recip -> dram ->
                    # bcast -> mul (psum direct), no U staging
                    for qc4 in range(4) if stop_after not in ("qkv", "exp") else []:
                        g = h // 2
                        pav = ps_av.tile([128, 512], F32, tag="avt",
                                         name="pavn2")
                        for kt in range(NKT):
                            nc.tensor.matmul(
                                pav[0:HD + 1, :],
                                v_sb[:, kt * (HPC * 65) + h * 65:
                                     kt * (HPC * 65) + (h + 1) * 65],
                                exp_tiles[kt][:, qc4 * 512:(qc4 + 1) * 512],
                                start=(kt == 0), stop=(kt == NKT - 1))
                        srow = nstage.tile([1, 512], F32, tag="srow",
                                           name="srow")
                        nc.scalar.copy(srow[0:1, :], pav[HD:HD + 1, :])
                        folded = small.tile([128, 4], F32, tag="folded",
                                            name="folded")
                        sap = srow[0:1, :]
                        nc.gpsimd.dma_start(
                            out=folded[:].rearrange("p j -> () p j"),
                            in_=bass.AP(tensor=sap.tensor, offset=sap.offset,
                                        ap=[[1, 1], [4, 128], [1, 4]]))
                        rcpf = small.tile([128, 4], F32, tag="rcpf",
                                          name="rcpf")
                        nc.vector.reciprocal(rcpf[:], folded[:])
                        d2 = dramp.tile([512], F32, tag="d2", name="d2")
                        nc.gpsimd.dma_start(
                            out=d2[:].rearrange("(p j) -> p j", j=4),
                            in_=rcpf[:])
                        bcast = bcastp.tile([64, 512], F32, tag="bcast",
                                            name="bcast")
                        d2ap = d2[:]
                        nc.gpsimd.dma_start(
                            out=bcast[:],
                            in_=bass.AP(tensor=d2ap.tensor, offset=d2ap.offset,
                                        ap=[[0, 64]] + list(d2ap.ap)))
                        nc.vector.tensor_mul(
                            uT_sb[po:po + HD,
                                  g * N + qc4 * 512: g * N + (qc4 + 1) * 512],
                            pav[0:HD, :], bcast[:])
                elif "formbn" in opts:
                    if stop_after not in ("qkv", "exp"):
                        g = h // 2
                        ustg = nstage.tile([64, N], BF, tag="ustg", name="ustg")
                        sums_sb = nstage.tile([65, N], F32, tag="sums",
                                              name="sums", bufs=1)
                        for qc4 in range(4):
                            pav = ps_av.tile([128, 512], F32, tag="avt",
                                             name="pavn")
                            for kt in range(NKT):
                                nc.tensor.matmul(
                                    pav[0:HD + 1, :],
                                    v_sb[:, kt * (HPC * 65) + h * 65:
                                         kt * (HPC * 65) + (h + 1) * 65],
                                    exp_tiles[kt][:, qc4 * 512:(qc4 + 1) * 512],
                                    start=(kt == 0), stop=(kt == NKT - 1))
                            if "nd" in opts:
                                nc.vector.tensor_copy(
                                    ustg[:, qc4 * 512:(qc4 + 1) * 512],
                                    pav[0:HD, :])
                            else:
                                nc.scalar.copy(
                                    ustg[:, qc4 * 512:(qc4 + 1) * 512],
                                    pav[0:HD, :])
                            if "nd2" in opts:
                                nc.vector.tensor_copy(
                                    sums_sb[HD:HD + 1,
                                            qc4 * 512:(qc4 + 1) * 512],
                                    pav[HD:HD + 1, :])
                            else:
                                nc.scalar.copy(
                                    sums_sb[HD:HD + 1,
                                            qc4 * 512:(qc4 + 1) * 512],
                                    pav[HD:HD + 1, :])
                        # fold sums [4,512] -> dram[2048] -> [128,16]
                        d1 = dramp.tile([N], F32, tag="d1", name="d1")
                        nc.sync.dma_start(
                            out=d1[:].rearrange("(o n) -> o n", o=1),
                            in_=sums_sb[HD:HD + 1, :])
                        folded = small.tile([128, 16], F32, tag="folded",
                                            name="folded")
                        nc.sync.dma_start(
                            out=folded[:],
                            in_=d1[:].rearrange("(p j) -> p j", j=16))
                        rcpf = small.tile([128, 16], F32, tag="rcpf",
                                          name="rcpf")
                        nc.vector.reciprocal(rcpf[:], folded[:])
                        d2 = dramp.tile([N], F32, tag="d2", name="d2")
                        nc.sync.dma_start(
                            out=d2[:].rearrange("(p j) -> p j", j=16),
                            in_=rcpf[:])
                        bcast = bcastp.tile([64, N], F32, tag="bcast",
                                            name="bcast")
                        d2ap = d2[:]
                        bcast_in = bass.AP(
                            tensor=d2ap.tensor, offset=d2ap.offset,
                            ap=[[0, 64]] + list(d2ap.ap))
                        nc.sync.dma_start(out=bcast[:], in_=bcast_in)
                        nc.vector.tensor_mul(
                            uT_sb[po:po + HD, g * N:(g + 1) * N],
                            ustg[:], bcast[:])
                elif "formb2" in opts:
                    # kt-outer form B: lhsT = v (stationary across 4 chunks)
                    if stop_after not in ("qkv", "exp"):
                        pavs = [ps_av.tile([128, 512], F32, tag="avt",
                                            name=f"pavb{j}")
                                for j in range(4)]
                        for kt in range(NKT):
                            for qc4 in range(4):
                                nc.tensor.matmul(
                                    pavs[qc4][0:HD + 1, :],
                                    v_sb[:, kt * (HPC * 65) + h * 65:
                                         kt * (HPC * 65) + (h + 1) * 65],
                                    exp_tiles[kt][:, qc4 * 512:(qc4 + 1) * 512],
                                    start=(kt == 0), stop=(kt == NKT - 1))
                        for qc4 in range(4):
                            nc.scalar.copy(
                                uT_sb[po:po + HD,
                                      (h // 2) * N + qc4 * 512:
                                      (h // 2) * N + (qc4 + 1) * 512],
                                pavs[qc4][0:HD, :])
                elif "formb" in opts:
                    # timing experiment: lhsT = v (65 cols), rhs = expS chunks
                    for qc4 in range(4) if stop_after not in ("qkv", "exp") else []:
                        pav = ps_av.tile([128, 512], F32, tag="avt")
                        for kt in range(NKT):
                            nc.tensor.matmul(
                                pav[0:HD + 1, :],
                                v_sb[:, kt * (HPC * 65) + h * 65:
                                     kt * (HPC * 65) + (h + 1) * 65],
                                exp_tiles[kt][:, qc4 * 512:(qc4 + 1) * 512],
                                start=(kt == 0), stop=(kt == NKT - 1))
                        # unnormalized copy (placeholder for timing)
                        nc.vector.tensor_copy(
                            uT_sb[po:po + HD,
                                  (h // 2) * N + qc4 * 512:
                                  (h // 2) * N + (qc4 + 1) * 512],
                            pav[0:HD, :])
                else:
                    for qt in range(NQT) if stop_after not in ("qkv", "exp") else []:
                        pav = ps_av.tile([128, HD + 1], F32, tag="avt")
                        for kt in range(NKT):
                            nc.tensor.matmul(
                                pav[:],
                                exp_tiles[kt][:, qt * 128:(qt + 1) * 128],
                                v_sb[:, kt * (HPC * 65) + h * 65:
                                     kt * (HPC * 65) + (h + 1) * 65],
                                start=(kt == 0), stop=(kt == NKT - 1))
                        rcp = small.tile([128, 1], F32)
                        nc.vector.reciprocal(rcp[:], pav[:, HD:HD + 1])
                        nc.vector.tensor_scalar_mul(
                            u_sb[:, qt * MPC + h * HD: qt * MPC + (h + 1) * HD],
                            pav[:, 0:HD], rcp[:])

            # ---- transpose U [N, MPC] -> UT [MPC, N] ----
            for ct in range(MPC // 128) if (stop_after not in ("qkv", "exp", "av") and not formb_mode) else []:
                for qt in range(NQT):
                    if "dmat" in opts:
                        nc.sync.dma_start(
                            out=uT_sb[:, ct * N + qt * 128: ct * N + (qt + 1) * 128],
                            in_=u_sb[:, qt * MPC + ct * 128: qt * MPC + (ct + 1) * 128],
                            transpose=True)
                        continue
                    if ps_t is None:
                        pt = ps_s.tile([128, 128], BF, tag="s")
                    else:
                        pt = ps_t.tile([128, 128], BF, tag="avt" if "s3" in opts else None)
                    nc.tensor.transpose(
                        pt[:],
                        u_sb[:, qt * MPC + ct * 128: qt * MPC + (ct + 1) * 128],
                        ident_sb[:])
                    nc.vector.tensor_copy(
                        uT_sb[:, ct * N + qt * 128: ct * N + (qt + 1) * 128], pt[:])

            # ---- partial projection: partial = U @ Wp_rows ----
            for qt in range(NQT) if stop_after not in ("qkv", "exp", "av", "trans") else []:
                for nk in range(2):
                    pp = ps_s.tile([128, 512], F32, tag="s")
                    for kc in range(4):
                        nc.tensor.matmul(
                            pp[:],
                            uT_sb[:, kc * N + qt * 128: kc * N + (qt + 1) * 128],
                            wp_sb[:, kc * C + nk * 512: kc * C + (nk + 1) * 512],
                            start=(kc == 0), stop=(kc == 3))
                    ost = ostage.tile([128, 512], F32)
                    if "projmix" in opts:
                        if (qt * 2 + nk) % 2 == 0:
                            nc.vector.tensor_copy(ost[:], pp[:])
                        else:
                            nc.scalar.copy(ost[:], pp[:])
                    elif "projdve" in opts:
                        nc.vector.tensor_copy(ost[:], pp[:])
                    else:
                        nc.scalar.copy(ost[:], pp[:])
                    nc.sync.dma_start(
                        out=partial[qt * 128:(qt + 1) * 128,
                                    nk * 512:(nk + 1) * 512],
                        in_=ost[:])

    nc.compile()
    return nc



def build_kernel_v2(reps=1, expu=22, pt_in="s", s_bufs=3):
    """Restructured: per-head-pair qT/kT/u tiles (early phase overlap),
    half-query expS units (early release), triple-buffered scores psum."""
    import contextlib
    nc = bacc.Bacc("TRN2", num_devices=8)
    xT = nc.dram_tensor("xT", [C, N], BF, kind="ExternalInput")
    xcT = nc.dram_tensor("xcT", [C, KP], BF, kind="ExternalInput")
    wq = nc.dram_tensor("wq", [C, MPC], BF, kind="ExternalInput")
    wk = nc.dram_tensor("wk", [C, MPC], BF, kind="ExternalInput")
    wv = nc.dram_tensor("wv", [C, MPC], BF, kind="ExternalInput")
    wp = nc.dram_tensor("wp", [MPC, C], BF, kind="ExternalInput")
    biasv = nc.dram_tensor("biasv", [KP], F32, kind="ExternalInput")
    partial = nc.dram_tensor("partial", [N, C], F32, kind="ExternalOutput")
    KC = C // 128

    with tile.TileContext(nc) as tc:
        with contextlib.ExitStack() as stack:
            persist = stack.enter_context(tc.tile_pool(name="persist", bufs=1))
            xtp = stack.enter_context(tc.tile_pool(name="xtp", bufs=10))
            expp = stack.enter_context(tc.tile_pool(name="exps", bufs=expu))
            small = stack.enter_context(tc.tile_pool(name="small", bufs=4))
            ostage = stack.enter_context(tc.tile_pool(name="ostage", bufs=3))
            if opts & {"formbn", "formbn2"}:
                nstage = stack.enter_context(tc.tile_pool(name="nstage", bufs=2))
                bcastp = stack.enter_context(tc.tile_pool(name="bcastp", bufs=2))
                dramp = stack.enter_context(
                    tc.tile_pool(name="dramp", bufs=3, space="DRAM"))
            ps_s = stack.enter_context(
                tc.tile_pool(name="ps_s", bufs=s_bufs, space="PSUM"))
            ps_av = stack.enter_context(
                tc.tile_pool(name="ps_av", bufs=2, space="PSUM"))
            if reps > 1:
                stack.enter_context(tc.For_i(0, reps, 1))

            if "wsplit" in opts:
                wqg = [persist.tile([128, MPC], BF, tag=f"wq{kc}",
                                    name=f"wq{kc}") for kc in range(KC)]
                wkg = [persist.tile([128, MPC], BF, tag=f"wk{kc}",
                                    name=f"wk{kc}") for kc in range(KC)]
                wvg = [persist.tile([128, MPC], BF, tag=f"wv{kc}",
                                    name=f"wv{kc}") for kc in range(KC)]
            else:
                wq_sb = persist.tile([128, KC * MPC], BF, tag="wq")
                wk_sb = persist.tile([128, KC * MPC], BF, tag="wk")
                wv_sb = persist.tile([128, KC * MPC], BF, tag="wv")
            wp_sb = persist.tile([128, 4 * C], BF, tag="wp")
            if "xsplit" in opts:
                xcg = [persist.tile([128, KP], BF, tag=f"xc{kc}",
                                    name=f"xc{kc}") for kc in range(KC)]
            else:
                xcT_sb = persist.tile([128, KC * KP], BF, tag="xcT")
            qTg = [persist.tile([128, N], BF, tag=f"qT{g}", name=f"qT{g}")
                   for g in range(4)]
            kTg = [persist.tile([128, KP], BF, tag=f"kT{g}", name=f"kT{g}")
                   for g in range(4)]
            ug = [persist.tile([128, NQT * 128], BF, tag=f"u{g}", name=f"u{g}")
                  for g in range(4)]
            v_sb = persist.tile([128, NKT * (HPC * (HD + 1))], BF, tag="v")
            uT_sb = persist.tile([128, 4 * N], BF, tag="uT")
            bias_sb = persist.tile([128, NKT], F32, tag="bias")
            ident_sb = persist.tile([128, 128], BF, tag="ident")
            make_identity(nc, ident_sb[:])

            nc.sync.dma_start(
                out=wq_sb[:].rearrange("p (kc m) -> p kc m", kc=KC),
                in_=wq.rearrange("(kc p) m -> p kc m", p=128))
            nc.sync.dma_start(
                out=wk_sb[:].rearrange("p (kc m) -> p kc m", kc=KC),
                in_=wk.rearrange("(kc p) m -> p kc m", p=128))
            nc.sync.dma_start(
                out=wv_sb[:].rearrange("p (kc m) -> p kc m", kc=KC),
                in_=wv.rearrange("(kc p) m -> p kc m", p=128))
            nc.sync.dma_start(
                out=wp_sb[:].rearrange("p (kc m) -> p kc m", kc=4),
                in_=wp.rearrange("(kc p) m -> p kc m", p=128))
            if "xsplit" in opts:
                if "wsplit" not in opts:
                    for kc in range(KC):
                        nc.sync.dma_start(
                            out=xcg[kc][:],
                            in_=xcT[kc * 128:(kc + 1) * 128, :])
            else:
                nc.sync.dma_start(
                    out=xcT_sb[:].rearrange("p (kc k) -> p kc k", kc=KC),
                    in_=xcT.rearrange("(kc p) k -> p kc k", p=128))
            nc.sync.dma_start(
                out=bias_sb[:], in_=biasv.rearrange("(kt p) -> p kt", p=128))
            for kt in range(NKT):
                nc.vector.memset(
                    v_sb[:, kt * (HPC * 65) + 64: (kt + 1) * (HPC * 65): 65], 1.0)

            # ---- k^T per head-pair ----
            for g in range(4):
                for qc in range(KP // 384):
                    pk = ps_s.tile([128, 384], F32, tag="s")
                    for kc in range(KC):
                        nc.tensor.matmul(
                            pk[:],
                            wk_sb[:, kc * MPC + g * 128: kc * MPC + (g + 1) * 128],
                            (xcg[kc][:, qc * 384:(qc + 1) * 384]
                             if "xsplit" in opts else
                             xcT_sb[:, kc * KP + qc * 384: kc * KP + (qc + 1) * 384]),
                            start=(kc == 0), stop=(kc == KC - 1))
                    nc.vector.tensor_copy(
                        kTg[g][:, qc * 384:(qc + 1) * 384], pk[:])

            # simpler: per g, per qc: load 8 x-tiles, matmul-accumulate
            for g in range(4):
                for qc in range(N // 512):
                    xts = []
                    for kc in range(KC):
                        xt = xtp.tile([128, 512], BF, tag="xt", name=f"xt{kc}")
                        nc.sync.dma_start(
                            out=xt[:], in_=xT[kc * 128:(kc + 1) * 128,
                                              qc * 512:(qc + 1) * 512])
                        xts.append(xt)
                    pq = ps_s.tile([128, 512], F32, tag="s")
                    for kc in range(KC):
                        nc.tensor.matmul(
                            pq[:],
                            wq_sb[:, kc * MPC + g * 128: kc * MPC + (g + 1) * 128],
                            xts[kc][:],
                            start=(kc == 0), stop=(kc == KC - 1))
                    nc.vector.tensor_copy(
                        qTg[g][:, qc * 512:(qc + 1) * 512], pq[:])
                if g == 0:
                    # ---- v projection (needed before AV of head pair 0) ----
                    for kt in range(NKT):
                        pv = ps_s.tile([128, MPC], F32, tag="s")
                        for kc in range(KC):
                            nc.tensor.matmul(
                                pv[:],
                                xcT_sb[:, kc * KP + kt * 128: kc * KP + (kt + 1) * 128],
                                (wvg[kc][:] if "wsplit" in opts else
                         wv_sb[:, kc * MPC: (kc + 1) * MPC]),
                                start=(kc == 0), stop=(kc == KC - 1))
                        vdst = v_sb[:, kt * (HPC * 65): (kt + 1) * (HPC * 65)]
                        vdst3 = vdst.rearrange("p (h d) -> p h d", h=HPC)[:, :, 0:HD]
                        psrc3 = pv.rearrange("p (h d) -> p h d", h=HPC)
                        nc.vector.tensor_copy(vdst3, psrc3)

            # ---- attention ----
            for h in range(HPC):
                g = h // 2
                po = (h % 2) * 64
                exp_tiles = {}
                for kt in range(NKT):
                    for qh in range(2):
                        et = expp.tile([128, 1024], BF, tag="e", name=f"e{kt}_{qh}")
                        ps = ps_s.tile([128, 1024], F32, tag="s")
                        for q2 in range(2):
                            nc.tensor.matmul(
                                ps[:, q2 * 512:(q2 + 1) * 512],
                                kTg[g][po:po + 64, kt * 128:(kt + 1) * 128],
                                qTg[g][po:po + 64,
                                       qh * 1024 + q2 * 512:
                                       qh * 1024 + (q2 + 1) * 512],
                                start=True, stop=True)
                        nc.scalar.activation(
                            et[:], ps[:],
                            mybir.ActivationFunctionType.Exp,
                            bias=bias_sb[:, kt:kt + 1], scale=0.125)
                        exp_tiles[(kt, qh)] = et
                for qh in range(2):
                    for qt8 in range(8):
                        qt = qh * 8 + qt8
                        pav = ps_av.tile([128, HD + 1], F32, tag="avt")
                        for kt in range(NKT):
                            nc.tensor.matmul(
                                pav[:],
                                exp_tiles[(kt, qh)][:, qt8 * 128:(qt8 + 1) * 128],
                                v_sb[:, kt * (HPC * 65) + h * 65:
                                     kt * (HPC * 65) + (h + 1) * 65],
                                start=(kt == 0), stop=(kt == NKT - 1))
                        rcp = small.tile([128, 1], F32)
                        nc.vector.reciprocal(rcp[:], pav[:, HD:HD + 1])
                        nc.vector.tensor_scalar_mul(
                            ug[g][:, qt * 128 + po: qt * 128 + po + HD],
                            pav[:, 0:HD], rcp[:])
                if h % 2 == 1:
                    # transpose this head pair: u_g -> uT columns
                    for qt in range(NQT):
                        if pt_in == "s":
                            pt = ps_s.tile([128, 128], BF, tag="s", name="pt")
                        else:
                            pt = ps_av.tile([128, 128], BF, tag="avt", name="pt")
                        nc.tensor.transpose(
                            pt[:], ug[g][:, qt * 128:(qt + 1) * 128],
                            ident_sb[:])
                        nc.vector.tensor_copy(
                            uT_sb[:, g * N + qt * 128: g * N + (qt + 1) * 128],
                            pt[:])

            # ---- partial projection ----
            for qt in range(NQT):
                for nk2 in range(2):
                    pp = ps_s.tile([128, 512], F32, tag="s")
                    for kc in range(4):
                        nc.tensor.matmul(
                            pp[:],
                            uT_sb[:, kc * N + qt * 128: kc * N + (qt + 1) * 128],
                            wp_sb[:, kc * C + nk2 * 512: kc * C + (nk2 + 1) * 512],
                            start=(kc == 0), stop=(kc == 3))
                    ost = ostage.tile([128, 512], F32)
                    nc.scalar.copy(ost[:], pp[:])
                    nc.sync.dma_start(
                        out=partial[qt * 128:(qt + 1) * 128,
                                    nk2 * 512:(nk2 + 1) * 512],
                        in_=ost[:])

    nc.compile()
    return nc


def build_kernel_v3(reps=1, opts=()):
    """Interleaved schedule: qkv-projection chains run as PE fillers inside
    the per-head attention loop so the PE never starves while ACT computes
    exps.  ACT does ONLY the 144 exps; sums/U-staging PSUM evictions go to
    the Pool (gpsimd) engine; xT tiles and softmax-fold DMAs ride the Pool
    DMA queue (cheap); output projection evictions on ACT (idle tail).

    PSUM: scores [128,1024]f32 x3 (6 banks) + av/filler/proj [128,512]f32
    x2 (2 banks) = 8 banks.
    """
    opts = set(opts)
    import contextlib

    nc = bacc.Bacc("TRN2", num_devices=8)
    xT = nc.dram_tensor("xT", [C, N], BF, kind="ExternalInput")
    xcT = nc.dram_tensor("xcT", [C, KP], BF, kind="ExternalInput")
    wq = nc.dram_tensor("wq", [C, MPC], BF, kind="ExternalInput")
    wk = nc.dram_tensor("wk", [C, MPC], BF, kind="ExternalInput")
    wv = nc.dram_tensor("wv", [C, MPC], BF, kind="ExternalInput")
    wp = nc.dram_tensor("wp", [MPC, C], BF, kind="ExternalInput")
    biasv = nc.dram_tensor("biasv", [KP], F32, kind="ExternalInput")
    ODT = BF if "obf16" in opts else F32
    partial = nc.dram_tensor("partial", [N, C], ODT, kind="ExternalOutput")
    KC = C // 128

    with tile.TileContext(nc) as tc:
        with contextlib.ExitStack() as stack:
            persist = stack.enter_context(tc.tile_pool(name="persist", bufs=1))
            expp = stack.enter_context(tc.tile_pool(
                name="exps", bufs=(14 if "e14" in opts else
                                   11 if "hnorm" in opts else 12)))
            ostage = stack.enter_context(tc.tile_pool(
                name="ostage", bufs=5 if "deep" in opts else 3))
            rcpp = stack.enter_context(tc.tile_pool(
                name="rcpp", bufs=4 if "deep" in opts else 2))
            bcastp = stack.enter_context(tc.tile_pool(name="bcastp", bufs=2))
            if opts & {"dmanorm", "hnorm"}:
                dramp = stack.enter_context(
                    tc.tile_pool(name="dramp", bufs=3, space="DRAM"))
            sbufs, avbufs = (2, 4) if "sb2av4" in opts else (3, 2)
            ps_s = stack.enter_context(
                tc.tile_pool(name="ps_s", bufs=sbufs, space="PSUM"))
            ps_av = stack.enter_context(
                tc.tile_pool(name="ps_av", bufs=avbufs, space="PSUM"))
            loop_cm = None
            if reps > 1:
                hints = ((mybir.EngineType.PE, mybir.EngineType.Activation,
                          mybir.EngineType.DVE, mybir.EngineType.Pool,
                          mybir.EngineType.SP)
                         if "hint" in opts else ())
                loop_cm = tc.For_i(0, reps, 1, hint_engines=hints)
                if "xrep" not in opts:
                    stack.enter_context(loop_cm)

            # ---- persistent SBUF ----
            wqg = [persist.tile([128, MPC], BF, tag=f"wq{kc}", name=f"wq{kc}")
                   for kc in range(KC)]
            wkg = [persist.tile([128, MPC], BF, tag=f"wk{kc}", name=f"wk{kc}")
                   for kc in range(KC)]
            wvg = [persist.tile([128, MPC], BF, tag=f"wv{kc}", name=f"wv{kc}")
                   for kc in range(KC)]
            wp_sb = persist.tile([128, 4 * C], BF, tag="wp")
            xcg = [persist.tile([128, KP], BF, tag=f"xc{kc}", name=f"xc{kc}")
                   for kc in range(KC)]
            xTg = [persist.tile([128, N], BF, tag=f"xT{kc}", name=f"xT{kc}")
                   for kc in range(KC)]
            qTg = [persist.tile([128, N], BF, tag=f"qT{g}", name=f"qT{g}")
                   for g in range(4)]
            kTg = [persist.tile([128, KP], BF, tag=f"kT{g}", name=f"kT{g}")
                   for g in range(4)]
            VW = (2 * HD if opts & {"penorm3", "penorm4"}
                  else HD + 1)  # per-head v width
            v_sb = persist.tile([128, NKT * HPC * VW], BF, tag="v")
            uTg = [persist.tile([128, N], BF, tag=f"uT{g}", name=f"uT{g}")
                   for g in range(4)]
            bias_sb = persist.tile([128, NKT], F32, tag="bias")
            if opts & {"penorm", "penorm2"}:
                ones_sb = persist.tile([65, HD], BF, tag="ones")
                nc.vector.memset(ones_sb[HD:HD + 1, :], 1.0)
            if "avA" in opts:
                ident_sb = persist.tile([128, 128], BF, tag="ident")
                make_identity(nc, ident_sb[:])
                ug = [persist.tile([128, N], BF, tag=f"u{g}", name=f"u{g}")
                      for g in range(4)]  # [q-part, qt*128 + po + d]

            # ---- input DMAs (order = need order) ----
            def emit_input_dmas():
                for kc in range(KC):
                    nc.sync.dma_start(out=wqg[kc][:],
                                      in_=wq[kc * 128:(kc + 1) * 128, :])
                    (nc.sync if "allsp" in opts else nc.gpsimd).dma_start(
                        out=xTg[kc][:],
                        in_=xT[kc * 128:(kc + 1) * 128, :])
                nc.sync.dma_start(
                    out=bias_sb[:],
                    in_=biasv.rearrange("(kt p) -> p kt", p=128))
                for kc in range(KC):
                    nc.sync.dma_start(out=wkg[kc][:],
                                      in_=wk[kc * 128:(kc + 1) * 128, :])
                    nc.sync.dma_start(out=xcg[kc][:],
                                      in_=xcT[kc * 128:(kc + 1) * 128, :])
                if "dmaspread" not in opts:
                    for kc in range(KC):
                        nc.sync.dma_start(out=wvg[kc][:],
                                          in_=wv[kc * 128:(kc + 1) * 128, :])
                    nc.sync.dma_start(
                        out=wp_sb[:].rearrange("p (kc m) -> p kc m", kc=4),
                        in_=wp.rearrange("(kc p) m -> p kc m", p=128))
            emit_input_dmas()
            for kt in range(NKT):
                if opts & {"penorm3", "penorm4"}:
                    vs = v_sb[:, kt * (HPC * VW): (kt + 1) * (HPC * VW)]
                    nc.vector.memset(
                        vs.rearrange("p (h two d) -> p h two d",
                                     h=HPC, two=2)[:, :, 1, :], 1.0)
                else:
                    nc.vector.memset(
                        v_sb[:, kt * (HPC * VW) + HD:
                             (kt + 1) * (HPC * VW): VW], 1.0)

            # ---- filler building blocks (each thunk = one PSUM chain) ----
            def qproj_chain(g, qc):
                def run():
                    pq = ps_av.tile([128, 512], F32, tag="avt", name="pq")
                    for kc in range(KC):
                        nc.tensor.matmul(
                            pq[:],
                            wqg[kc][:, g * 128:(g + 1) * 128],
                            xTg[kc][:, qc * 512:(qc + 1) * 512],
                            start=(kc == 0), stop=(kc == KC - 1))
                    nc.vector.tensor_copy(
                        qTg[g][:, qc * 512:(qc + 1) * 512], pq[:])
                return run

            NKCH = (KP + 383) // 384  # kproj chains (last may be short)

            def kproj_chain(g, qc):
                kw = min(384, KP - qc * 384)

                def run():
                    pk = ps_av.tile([128, kw], F32, tag="avt", name="pk")
                    for kc in range(KC):
                        nc.tensor.matmul(
                            pk[:],
                            wkg[kc][:, g * 128:(g + 1) * 128],
                            xcg[kc][:, qc * 384: qc * 384 + kw],
                            start=(kc == 0), stop=(kc == KC - 1))
                    nc.vector.tensor_copy(
                        kTg[g][:, qc * 384: qc * 384 + kw], pk[:])
                return run

            def vproj_chain(kt):
                def run():
                    pv = ps_av.tile([128, MPC], F32, tag="avt", name="pv")
                    for kc in range(KC):
                        nc.tensor.matmul(
                            pv[:],
                            xcg[kc][:, kt * 128:(kt + 1) * 128], wvg[kc][:],
                            start=(kc == 0), stop=(kc == KC - 1))
                    vdst = v_sb[:, kt * (HPC * VW): (kt + 1) * (HPC * VW)]
                    vdst3 = vdst.rearrange("p (h d) -> p h d", h=HPC)[:, :, 0:HD]
                    nc.vector.tensor_copy(
                        vdst3, pv.rearrange("p (h d) -> p h d", h=HPC))
                return run

            # wait: vproj pv is [128, 512] = MPC -> 1 bank (2KB) OK

            # filler plan: vproj must finish before AV(h0); kproj/qproj(g)
            # before scores(2g) i.e. fillers at heads < 2g.
            fillers = {h: [] for h in range(HPC)}
            fillers[0] = [vproj_chain(kt) for kt in range(NKT)]
            fillers[1] = ([kproj_chain(1, qc) for qc in range(NKCH)]
                          + [qproj_chain(1, qc) for qc in range(4)])
            fillers[2] = ([kproj_chain(2, qc) for qc in range(NKCH)]
                          + [qproj_chain(2, qc) for qc in range(2)])
            fillers[3] = [qproj_chain(2, qc) for qc in range(2, 4)]
            fillers[4] = ([kproj_chain(3, qc) for qc in range(NKCH)]
                          + [qproj_chain(3, qc) for qc in range(2)])
            fillers[5] = [qproj_chain(3, qc) for qc in range(2, 4)]

            if "dmaspread" in opts:
                for kc in range(KC):
                    nc.sync.dma_start(out=wvg[kc][:],
                                      in_=wv[kc * 128:(kc + 1) * 128, :])

            # ---- phase A: q/k projections for pair 0 ----
            xrep = "xrep" in opts and reps > 1
            phase_a = ([qproj_chain(0, qc) for qc in range(4)]
                       + [kproj_chain(0, qc) for qc in range(NKCH)])
            if not xrep:
                for f in phase_a:
                    f()
            if "p_nofill" in opts:
                for hh in range(HPC):
                    for f in fillers[hh]:
                        f()
                    fillers[hh] = []

            # ---- attention, one head at a time ----
            pending_norm = []  # (g, po, ustg, rcrow) awaiting pbc+mul

            def flush_norm():
                while pending_norm:
                    pg, ppo, pustg, prcrow = pending_norm.pop(0)
                    for qc4 in range(4):
                        pbc = ps_av.tile([64, 512], F32, tag="avt",
                                         name="pbc")
                        nc.tensor.matmul(
                            pbc[:], ones_sb[HD:HD + 1, :],
                            prcrow[HD:HD + 1, qc4 * 512:(qc4 + 1) * 512],
                            start=True, stop=True)
                        nc.vector.tensor_mul(
                            uTg[pg][ppo:ppo + HD,
                                    qc4 * 512:(qc4 + 1) * 512],
                            pustg[:, qc4 * 512:(qc4 + 1) * 512], pbc[:])

            if xrep:
                for f in phase_a:
                    f()
                stack.enter_context(loop_cm)
                # in-loop copy of the input DMAs (content identical each
                # rep; WAR deps order them after the prologue/tail reads)
                emit_input_dmas()

            def score_tile(h, kt):
                g, po = h // 2, (h % 2) * 64
                et = expp.tile([128, N], BF, tag="e", name=f"e{kt}")
                for qh in range(2):
                    ps = ps_s.tile([128, 1024], F32, tag="s")
                    for q2 in range(2):
                        nc.tensor.matmul(
                            ps[:, q2 * 512:(q2 + 1) * 512],
                            kTg[g][po:po + 64,
                                   kt * 128:(kt + 1) * 128],
                            qTg[g][po:po + 64,
                                   qh * 1024 + q2 * 512:
                                   qh * 1024 + (q2 + 1) * 512],
                            start=True, stop=True)
                    nc.scalar.activation(
                        et[:, qh * 1024:(qh + 1) * 1024], ps[:],
                        mybir.ActivationFunctionType.Exp,
                        bias=bias_sb[:, kt:kt + 1], scale=0.125)
                return et

            def av_chunk3(h, exp_tiles, qc4):
                g, po = h // 2, (h % 2) * 64
                pav = ps_av.tile([128, 512], F32, tag="avt", name="pav")
                for kt in range(NKT):
                    nc.tensor.matmul(
                        pav[:],
                        v_sb[:, kt * (HPC * VW) + h * VW:
                             kt * (HPC * VW) + (h + 1) * VW],
                        exp_tiles[kt][:, qc4 * 512:(qc4 + 1) * 512],
                        start=(kt == 0), stop=(kt == NKT - 1))
                rcp64 = rcpp.tile([64, 512], BF, tag="rcp64", name="rcp64")
                with nc.allow_low_precision(
                        reason="1/denom in bf16: 0.2% on softmax scale, "
                               "within tolerance"):
                    if "arecip" in opts:
                        nc.scalar.activation(
                            rcp64[:], pav[HD:2 * HD, :],
                            mybir.ActivationFunctionType.Reciprocal)
                    else:
                        nc.vector.reciprocal(rcp64[:], pav[HD:2 * HD, :])
                mul_eng = nc.gpsimd if "poolmul" in opts else nc.vector
                mul_eng.tensor_mul(
                    uTg[g][po:po + HD, qc4 * 512:(qc4 + 1) * 512],
                    pav[0:HD, :], rcp64[:])

            lacell = "lacell" in opts and "penorm3" in opts
            for h in range(HPC):
                if h == 7 and lacell:
                    continue  # handled inside h == 6 cell
                if h == 4 and "dmaspread" in opts:
                    nc.sync.dma_start(
                        out=wp_sb[:].rearrange("p (kc m) -> p kc m", kc=4),
                        in_=wp.rearrange("(kc p) m -> p kc m", p=128))
                g = h // 2
                po = (h % 2) * 64
                if h == 6 and lacell:
                    # software-pipeline the fillerless last pair: head 7
                    # scores interleave with head 6 AV chunks (14 exp
                    # tiles peak)
                    et6 = [score_tile(6, kt) for kt in range(NKT)]
                    av_chunk3(6, et6, 0)
                    av_chunk3(6, et6, 1)
                    et7 = [score_tile(7, kt) for kt in range(5)]
                    av_chunk3(6, et6, 2)
                    av_chunk3(6, et6, 3)
                    et7 += [score_tile(7, kt) for kt in range(5, NKT)]
                    for qc4 in range(4):
                        av_chunk3(7, et7, qc4)
                    continue
                exp_tiles = [score_tile(h, kt) for kt in range(NKT)]

                if "penorm2" in opts:
                    flush_norm()
                if "p_nofill" not in opts:
                    for f in fillers[h]:
                        f()

                if "p_noav" in opts:
                    continue
                if "penorm4" in opts:
                    # like penorm3 but one fused divide per chunk
                    for qc4 in range(4):
                        pav = ps_av.tile([128, 512], F32, tag="avt",
                                         name="pav")
                        for kt in range(NKT):
                            nc.tensor.matmul(
                                pav[:],
                                v_sb[:, kt * (HPC * VW) + h * VW:
                                     kt * (HPC * VW) + (h + 1) * VW],
                                exp_tiles[kt][:, qc4 * 512:(qc4 + 1) * 512],
                                start=(kt == 0), stop=(kt == NKT - 1))
                        nc.vector.tensor_tensor(
                            uTg[g][po:po + HD, qc4 * 512:(qc4 + 1) * 512],
                            pav[0:HD, :], pav[HD:2 * HD, :],
                            mybir.AluOpType.divide)
                    continue
                if "penorm3" in opts:
                    # ones block replicated 64x in stationary v: PE emits
                    # U on parts 0-63 and D broadcast on parts 64-127.
                    # Full-width recip + mul, no single-partition DVE ops.
                    for qc4 in range(4):
